# revision 20
# baseline (speedup 1.0000x reference)
"""DualLaplacianBlock Trainium2 kernel (v4 — slot-major fused pipeline).

Same math/host contract as v3 (fp8 DoubleRow hi/lo chains, K_g==0
specialization, W_O@W_V fusion, (batch, parity) sharding), but the device
program is restructured into a slot-major software pipeline so the PE never
waits on the vector engines:

    c0 G3 c1 G2 c2 G1 c3 G0 KV3 KV2 P4(pr1) KV1 KV0 P4(pr0)

where c<k> is a 512-column z-projection chunk, G<s> the slot-s gram (+deg),
KV<s> = K@h for slot s, and P4 the Wvo^T output GEMM per column half.  Each
slot's normalize/combine/cast work (V<s>) runs on DVE/Act/Pool in the shadow
of the next PE phase; the per-slot deg tail (last two m-tiles) is deferred
into the following PE phase to keep the PE queue from stalling on relu/mask
latency.
"""

import sys

if "/opt/trn_rl_repo" not in sys.path:
    sys.path.insert(0, "/opt/trn_rl_repo")

from contextlib import ExitStack

import ml_dtypes
import numpy as np

import concourse.bass as bass
import concourse.tile as tile
from concourse import bacc, mybir
from concourse.bass_utils import run_bass_kernel_spmd
from concourse.masks import make_identity

F32 = mybir.dt.float32
F32R = mybir.dt.float32r
BF16 = mybir.dt.bfloat16
FP8 = mybir.dt.float8e4
FP8L = mybir.dt.float8e5
AF = mybir.ActivationFunctionType
OP = mybir.AluOpType
PM = mybir.MatmulPerfMode

E4NP = ml_dtypes.float8_e4m3
E5NP = ml_dtypes.float8_e5m2

B, N, D = 4, 2048, 1024
P = 128
ET = D // P                      # 8 e-tiles (also d-tiles)
NSLOT = 4
EXT = [2048, 1536, 1024, 512]    # slot column extents (pattern, all cores)
MT = [e // P for e in EXT]       # m-tiles per slot: 16, 12, 8, 4
OFF = [0, 16, 28, 36]            # slot plane offsets in the K tiles
OWNW = 256                       # own columns per slot
EPS = 1e-8
SW = 32.0                        # host weight pre-scale (power of two)

TRACE = False          # set by test.py for profiling runs
LAST_RESULTS = [None]  # BassKernelResults stash for test.py


def _build_program():
    nc = bacc.Bacc("TRN2", target_bir_lowering=False, debug=False, num_devices=8)

    htHi_d = nc.dram_tensor("htHi", [D, N], FP8, kind="ExternalInput")
    htLo_d = nc.dram_tensor("htLo", [D, N], FP8, kind="ExternalInput")
    hrHi_d = nc.dram_tensor("hrHi", [N, D], FP8, kind="ExternalInput")
    hrLo_d = nc.dram_tensor("hrLo", [N, D], FP8, kind="ExternalInput")
    wlHi_d = nc.dram_tensor("wlHi", [D, D], FP8, kind="ExternalInput")
    wlLo_d = nc.dram_tensor("wlLo", [D, D], FP8, kind="ExternalInput")
    wvoHi_d = nc.dram_tensor("wvoHi", [D, D], FP8, kind="ExternalInput")
    wvoLo_d = nc.dram_tensor("wvoLo", [D, D], FP8, kind="ExternalInput")
    maskT_d = nc.dram_tensor("maskT", [NSLOT, 512, OWNW], BF16, kind="ExternalInput")
    wlg_d = nc.dram_tensor("wlg", [1, 1], F32, kind="ExternalInput")
    yT_d = nc.dram_tensor("yT", [D, 4 * OWNW], BF16, kind="ExternalOutput")

    def dview(t):  # [R, C] dram -> [128, R//128, C] view
        return t[:].rearrange("(o p) c -> p o c", p=P)

    with tile.TileContext(nc) as tc, ExitStack() as ctx:
        glob = ctx.enter_context(tc.tile_pool(name="glob", bufs=1))

        wl = glob.tile([1, 1], F32, tag="wl")
        wlb = glob.tile([P, 1], F32, tag="wlb")  # wl on all partitions

        onesf = glob.tile([P, 1], F32, tag="onesf")
        nc.vector.memset(onesf[:], 1.0)
        ones = glob.tile([P, 1], F32R, tag="ones")
        nc.scalar.activation(ones[:], onesf[:], AF.Copy)
        onesp = glob.tile([P, 2, P], FP8, tag="onesp")  # deg stationary
        nc.vector.memset(onesp[:], 1.0)
        ident = glob.tile([P, P], F32, tag="ident")
        make_identity(nc, ident[:])

        diagl = glob.tile([P, 16], F32, tag="diagl")  # |z~_m|^2 per m-tile
        rsl = glob.tile([P, 16], F32, tag="rsl")      # 1/|z~_m|
        msk = glob.tile([P, 16, OWNW], BF16, tag="msk")

        junkp = ctx.enter_context(tc.tile_pool(name="junkp", bufs=2))
        smp = ctx.enter_context(tc.tile_pool(name="smp", bufs=2))
        ypool = ctx.enter_context(tc.tile_pool(name="ypool", bufs=4))

        # big SBUF operands
        wpool = ctx.enter_context(tc.tile_pool(name="wpool", bufs=1))
        wlHi = wpool.tile([P, ET, D], FP8, tag="wHi")
        wlLo = wpool.tile([P, ET, D], FP8, tag="wLo")
        hpool = ctx.enter_context(tc.tile_pool(name="hpool", bufs=1))
        htHi = hpool.tile([P, ET, N], FP8, tag="htHi")
        htLo = hpool.tile([P, ET, N], FP8, tag="htLo")
        zpool = ctx.enter_context(tc.tile_pool(name="zpool", bufs=1))
        zlHi = zpool.tile([P, ET, N], FP8, tag="zlHi")
        zlLo = zpool.tile([P, ET, N], FP8, tag="zlLo")
        hrp = ctx.enter_context(tc.tile_pool(name="hrp", bufs=1))
        hrHi = hrp.tile([P, 16, D], FP8, tag="hrHi")
        hrLo = hrp.tile([P, 16, D], FP8, tag="hrLo")
        apool = ctx.enter_context(tc.tile_pool(name="apool", bufs=1))
        A_even = apool.tile([P, 16, OWNW], F32R, tag="Aeven")  # slots 0, 2
        A_odd = apool.tile([P, 12, OWNW], F32R, tag="Aodd")    # slots 1, 3
        kpool = ctx.enter_context(tc.tile_pool(name="kpool", bufs=1))
        kHi = kpool.tile([P, 40, OWNW], FP8, tag="kHi")
        kLo = kpool.tile([P, 40, OWNW], FP8L, tag="kLo")
        opool = ctx.enter_context(tc.tile_pool(name="opool", bufs=1))
        o2Hi = opool.tile([P, ET, 4 * OWNW], FP8, tag="o2Hi")
        o2Lo = opool.tile([P, ET, 4 * OWNW], FP8, tag="o2Lo")

        # PSUM: 2 (big: pz/pd/py) + 3 (pkv) + 2 (pg) + 1 (pdl) = 8 banks.
        # pg/pdl are allocated last (top of the PSUM stack) so they can be
        # released after the last gram/deg use and their banks recycled for
        # the dedicated P4(pr=0) pipeline pool.
        bigp = ctx.enter_context(tc.tile_pool(name="bigp", bufs=2, space="PSUM"))
        pkvp = ctx.enter_context(tc.tile_pool(name="pkvp", bufs=3, space="PSUM"))
        gram_psum = ExitStack()
        pgp = gram_psum.enter_context(tc.tile_pool(name="pgp", bufs=2, space="PSUM"))
        pdlp = gram_psum.enter_context(tc.tile_pool(name="pdlp", bufs=1, space="PSUM"))

        def A_of(s):
            return A_even if s % 2 == 0 else A_odd

        # ---------------- input DMA stream (consumption order) -----------
        h0, h1 = slice(0, 512), slice(512, 1024)
        c4s = [slice(c * 512, (c + 1) * 512) for c in range(4)]
        nc.sync.dma_start(wlHi[:, 0:2, h0], dview(wlHi_d)[:, 0:2, h0])
        nc.sync.dma_start(htHi[:, :, c4s[0]], dview(htHi_d)[:, :, c4s[0]])
        nc.sync.dma_start(wlHi[:, 2:8, h0], dview(wlHi_d)[:, 2:8, h0])
        nc.sync.dma_start(wlLo[:, :, h0], dview(wlLo_d)[:, :, h0])
        nc.sync.dma_start(htLo[:, :, c4s[0]], dview(htLo_d)[:, :, c4s[0]])
        nc.sync.dma_start(wlHi[:, :, h1], dview(wlHi_d)[:, :, h1])
        nc.sync.dma_start(wlLo[:, :, h1], dview(wlLo_d)[:, :, h1])
        nc.sync.dma_start(htHi[:, :, c4s[1]], dview(htHi_d)[:, :, c4s[1]])
        nc.sync.dma_start(htLo[:, :, c4s[1]], dview(htLo_d)[:, :, c4s[1]])
        nc.sync.dma_start(htHi[:, :, c4s[2]], dview(htHi_d)[:, :, c4s[2]])
        nc.sync.dma_start(htLo[:, :, c4s[2]], dview(htLo_d)[:, :, c4s[2]])
        nc.sync.dma_start(htHi[:, :, c4s[3]], dview(htHi_d)[:, :, c4s[3]])
        nc.sync.dma_start(htLo[:, :, c4s[3]], dview(htLo_d)[:, :, c4s[3]])
        nc.sync.dma_start(wl[:], wlg_d[:])
        nc.gpsimd.partition_broadcast(wlb[:], wl[:])
        nc.sync.dma_start(
            msk[:], maskT_d[:].rearrange("s (t p) n -> p (s t) n", p=P))
        nc.sync.dma_start(hrHi[:], dview(hrHi_d))
        nc.sync.dma_start(hrLo[:], dview(hrLo_d))
        # wvo loads are emitted after P1c3 (tag reuse of the wl slots)

        # deferred deg-matmul closures (flushed into the next PE phase)
        pending_degs = []

        def flush_degs():
            while pending_degs:
                pending_degs.pop(0)()

        # ---------------- phase-1 chunk: z projection + diag --------------
        def emit_p1_pair01(c):
            # head special-case: run the (wlHi, htHi) chains of et0 AND et1
            # before et0's Lo chains, so the PE has work while the wlLo /
            # htLo DMAs are still in flight
            cs = c4s[c]
            chains = ((wlHi, htHi), (wlLo, htHi), (wlHi, htLo))
            pzs = [bigp.tile([P, 512], F32, tag="big", name=f"pz01_{e}")
                   for e in range(2)]
            for (ci, et) in ((0, 0), (0, 1), (1, 0), (2, 0), (1, 1), (2, 1)):
                wa, hb = chains[ci]
                es = slice(et * P, (et + 1) * P)
                for dp in range(4):
                    nc.tensor.matmul(
                        pzs[et][:], wa[:, 2 * dp:2 * dp + 2, es],
                        hb[:, 2 * dp:2 * dp + 2, cs],
                        start=(ci == 0 and dp == 0),
                        stop=(ci == 2 and dp == 3),
                        perf_mode=PM.DoubleRow,
                        skip_group_check=True)
            for et in range(2):
                nc.scalar.copy(zlHi[:, et, cs], pzs[et][:])
                nc.vector.tensor_sub(zlLo[:, et, cs], pzs[et][:],
                                     zlHi[:, et, cs])

        def emit_p1_et(c, et):
            cs = c4s[c]
            es = slice(et * P, (et + 1) * P)
            pz = bigp.tile([P, 512], F32, tag="big")
            chains = ((wlHi, htHi), (wlLo, htHi), (wlHi, htLo))
            for ci, (wa, hb) in enumerate(chains):
                for dp in range(4):
                    nc.tensor.matmul(
                        pz[:], wa[:, 2 * dp:2 * dp + 2, es],
                        hb[:, 2 * dp:2 * dp + 2, cs],
                        start=(ci == 0 and dp == 0),
                        stop=(ci == 2 and dp == 3),
                        perf_mode=PM.DoubleRow,
                        skip_group_check=True)
            nc.scalar.copy(zlHi[:, et, cs], pz[:])
            nc.vector.tensor_sub(zlLo[:, et, cs], pz[:], zlHi[:, et, cs])

        def emit_p1_diag(c):
            for mt4 in range(4):
                gmt = c * 4 + mt4
                ms = slice(gmt * P, (gmt + 1) * P)
                pd = bigp.tile([P, 512], F32, tag="big")
                chains = ((zlHi, zlHi), (zlLo, zlHi), (zlHi, zlLo))
                for ci, (za, zb) in enumerate(chains):
                    for ep in range(4):
                        e2 = slice(2 * ep, 2 * ep + 2)
                        nc.tensor.matmul(
                            pd[:, 0:P], za[:, e2, ms], zb[:, e2, ms],
                            start=(ci == 0 and ep == 0),
                            stop=(ci == 2 and ep == 3),
                            perf_mode=PM.DoubleRow,
                            skip_group_check=True)
                junk = junkp.tile([P, P], F32, tag="junk")
                nc.vector.tensor_mul(junk[:], pd[:, 0:P], ident[:])
                nc.vector.reduce_sum(diagl[:, gmt:gmt + 1], junk[:],
                                     axis=mybir.AxisListType.X)
            cc = slice(c * 4, c * 4 + 4)
            nc.scalar.activation(rsl[:, cc], diagl[:, cc], AF.Sqrt)
            nc.vector.tensor_scalar(rsl[:, cc], rsl[:, cc], SW * EPS, None,
                                    OP.max)
            nc.vector.reciprocal(rsl[:, cc], rsl[:, cc])
            # extra 1/32: the A tiles stay unnormalized until the KV psum
            # rescale, so pull them into fp8e4m3 range here (the global
            # factor cancels in row normalization; dl's constant restores
            # the absolute output scale)
            nc.vector.tensor_scalar(rsl[:, cc], rsl[:, cc], 1.0 / 32.0, None,
                                    OP.mult)

        # ---------------- slot gram + cast + deg ---------------------------
        # A is cast to fp8 hi/lo UNNORMALIZED right after relu/mask (the
        # 1/32 in rsl keeps it in e4m3 range); row normalization is applied
        # later as a single psum rescale per KV group.  deg accumulates from
        # the fp8 casts with DoubleRow pairs (half the PE time of the f32r
        # row-sum, and it frees the A f32 tiles early).
        def emit_G(s):
            at = A_of(s)
            own = slice(EXT[s] - OWNW, EXT[s])
            pdl = pdlp.tile([P, OWNW], F32, tag="pdl", name=f"pdl{s}")

            def emit_deg(g0):
                ks = slice(OFF[s] + g0, OFF[s] + g0 + 2)
                for ci, src in enumerate((kHi, kLo)):
                    nc.tensor.matmul(
                        pdl[:], onesp[:, 0:2, 0:P], src[:, ks, :],
                        start=(g0 == 0 and ci == 0),
                        stop=(g0 == MT[s] - 2 and ci == 1),
                        perf_mode=PM.DoubleRow,
                        skip_group_check=True)

            for gmt in range(MT[s]):
                ms = slice(gmt * P, (gmt + 1) * P)
                pg = pgp.tile([P, OWNW], F32, tag="pg")
                chains = ((zlHi, zlHi), (zlLo, zlHi), (zlHi, zlLo))
                for ci, (za, zb) in enumerate(chains):
                    for ep in range(4):
                        e2 = slice(2 * ep, 2 * ep + 2)
                        nc.tensor.matmul(
                            pg[:], za[:, e2, ms], zb[:, e2, own],
                            start=(ci == 0 and ep == 0),
                            stop=(ci == 2 and ep == 3),
                            perf_mode=PM.DoubleRow,
                            skip_group_check=True)
                nc.scalar.activation(at[:, gmt, :], pg[:], AF.Relu,
                                     scale=rsl[:, gmt:gmt + 1])
                if gmt >= MT[s] - 4:
                    bi = 4 * s + gmt - (MT[s] - 4)
                    nc.gpsimd.tensor_mul(at[:, gmt, :], at[:, gmt, :],
                                         msk[:, bi, :])
                if gmt % 2 == 1:
                    g0 = gmt - 1
                    ks = slice(OFF[s] + g0, OFF[s] + g0 + 2)
                    asl = slice(g0, g0 + 2)
                    nc.scalar.copy(kHi[:, ks, :], at[:, asl, :])
                    nc.vector.tensor_sub(kLo[:, ks, :], at[:, asl, :],
                                         kHi[:, ks, :])
                    if g0 >= 2:
                        emit_deg(g0 - 2)
            pending_degs.append(lambda g=MT[s] - 2: emit_deg(g))
            return pdl

        # ---------------- slot normalize (dinv broadcast only) ------------
        def emit_V(s, pdl):
            dlb = smp.tile([P, OWNW], F32, tag=f"dlb{s}", name=f"dlb{s}")
            nc.vector.tensor_scalar(dlb[:], pdl[:], EPS, None, OP.max)
            nc.vector.reciprocal(dlb[:], dlb[:])
            nc.vector.tensor_scalar(dlb[:], dlb[:], wlb[:], SW, OP.mult,
                                    OP.mult)
            return dlb

        # ---------------- slot KV: o2[:, :, scol] = (K_s @ h)^T -----------
        def emit_KV(s, dlb, flush_after=None):
            scol = slice(s * OWNW, (s + 1) * OWNW)
            for eh in range(2):
                for e2 in range(4):
                    et = eh * 4 + e2
                    es = slice(et * P, (et + 1) * P)
                    pkv = pkvp.tile([P, OWNW], F32, tag="pkv")
                    chains = ((hrHi, kHi), (hrLo, kHi), (hrHi, kLo))
                    for ci, (ha, kt) in enumerate(chains):
                        for gp in range(0, MT[s], 2):
                            nc.tensor.matmul(
                                pkv[:], ha[:, gp:gp + 2, es],
                                kt[:, OFF[s] + gp:OFF[s] + gp + 2, :],
                                start=(ci == 0 and gp == 0),
                                stop=(ci == 2 and gp == MT[s] - 2),
                                perf_mode=PM.DoubleRow,
                                skip_group_check=True)
                    # row normalization: one psum rescale per group
                    nc.vector.tensor_mul(pkv[:], pkv[:], dlb[:])
                    nc.scalar.copy(o2Hi[:, et, scol], pkv[:])
                    nc.vector.tensor_sub(o2Lo[:, et, scol], pkv[:],
                                         o2Hi[:, et, scol])
                    if flush_after is not None and eh == 0 and e2 == flush_after:
                        flush_degs()

        # ---------------- output half: y = out2 @ Wvo^T -------------------
        def emit_P4(pr, wvoHi, wvoLo, pool=None):
            # pr=0 runs with a dedicated 3-buffer pool (recycled pg/pdl
            # banks) in 256-wide groups: deeper psum pipelining and a short
            # drain for the kernel tail; y DMAs stay 512 wide.
            fine = pool is not None
            w = 256 if fine else 512
            grp = 2 if fine else 1   # e2t planes per y DMA
            oc2 = slice(pr * 512, (pr + 1) * 512)
            yt = None
            for e2t in range(ET):
                if e2t % grp == 0:
                    yt = ypool.tile([P, grp, 512], BF16, tag=f"yt{grp}")
                for half in range(512 // w):
                    ocol = slice(pr * 512 + half * w, pr * 512 + (half + 1) * w)
                    if fine:
                        py = pool.tile([P, w], F32, tag="pyf",
                                       name=f"py{pr}_{e2t}_{half}")
                    else:
                        py = bigp.tile([P, w], F32, tag="big",
                                       name=f"py{pr}_{e2t}")
                    chains = ((wvoHi, o2Hi), (wvoLo, o2Hi), (wvoHi, o2Lo))
                    for ci, (wa, ob) in enumerate(chains):
                        for dp in range(4):
                            d2 = slice(2 * dp, 2 * dp + 2)
                            nc.tensor.matmul(
                                py[:], wa[:, d2, e2t * P:(e2t + 1) * P],
                                ob[:, d2, ocol],
                                start=(ci == 0 and dp == 0),
                                stop=(ci == 2 and dp == 3),
                                perf_mode=PM.DoubleRow,
                                skip_group_check=True)
                    nc.scalar.mul(
                        yt[:, e2t % grp, half * w:(half + 1) * w], py[:],
                        1.0 / (32.0 * SW))
                if e2t % grp == grp - 1:
                    e0 = e2t - grp + 1
                    nc.sync.dma_start(
                        dview(yT_d)[:, e0:e2t + 1, oc2], yt[:])

        # ================= emission sequence ==============================
        emit_p1_pair01(0)
        for et in range(2, ET):
            emit_p1_et(0, et)
        emit_p1_diag(0)

        pdl3 = emit_G(3)

        for et in range(ET):
            emit_p1_et(1, et)
            if et == 0:
                flush_degs()
        emit_p1_diag(1)
        dlb3 = emit_V(3, pdl3)

        pdl2 = emit_G(2)

        for et in range(ET):
            emit_p1_et(2, et)
            if et == 0:
                flush_degs()
        emit_p1_diag(2)
        dlb2 = emit_V(2, pdl2)

        pdl1 = emit_G(1)

        for et in range(ET):
            emit_p1_et(3, et)
            if et == 0:
                flush_degs()
        emit_p1_diag(3)
        dlb1 = emit_V(1, pdl1)

        pdl0 = emit_G(0)

        # wl slots are dead after P1c3: reuse them for wvo (WAR-tracked)
        wvoHi = wpool.tile([P, ET, D], FP8, tag="wHi", name="wvoHi")
        nc.sync.dma_start(wvoHi[:], dview(wvoHi_d))
        wvoLo = wpool.tile([P, ET, D], FP8, tag="wLo", name="wvoLo")
        nc.sync.dma_start(wvoLo[:], dview(wvoLo_d))

        emit_KV(3, dlb3, flush_after=1)
        dlb0 = emit_V(0, pdl0)
        emit_KV(2, dlb2)
        emit_P4(1, wvoHi, wvoLo)
        emit_KV(1, dlb1)
        emit_KV(0, dlb0)
        gram_psum.close()  # pg/pdl banks -> P4(pr=0) pipeline pool
        pyfp = ctx.enter_context(tc.tile_pool(name="pyfp", bufs=3,
                                              space="PSUM"))
        emit_P4(0, wvoHi, wvoLo, pool=pyfp)

    nc.compile()
    return nc


_PROGRAM = None


def _get_program():
    global _PROGRAM
    if _PROGRAM is None:
        _PROGRAM = _build_program()
    return _PROGRAM


def _posmap(core):
    """Device position -> global sequence row for this core."""
    p = core % 2
    q = np.arange(N)
    if p == 0:
        return q
    return (q // 512) * 512 + (q % 512 + 256) % 512


def _hilo(x, lot=E4NP):
    hi = np.asarray(x).astype(E4NP)
    lo = (x - hi.astype(np.float32)).astype(lot)
    return hi, lo


def _g_path_is_zero(h, W_grav, log_sigma, mask_c):
    """Exact f32 check that exp(-d2/(2 sigma^2)) == 0 for all masked pairs."""
    sigma = np.exp(np.float32(log_sigma)).astype(np.float32)
    thresh = np.float32(-110.0) * (2.0 * sigma * sigma)
    WgT = np.asarray(W_grav, np.float32).T
    for b in range(B):
        zg = np.asarray(h[b], np.float32) @ WgT
        sq = np.einsum("nd,nd->n", zg, zg)
        d2 = sq[:, None] + sq[None, :] - 2.0 * (zg @ zg.T)
        if (d2[mask_c > 0] + thresh < 0).any():
            return False
    return True


def _make_in_maps(h, W_lang, Wvo, gate_logit, mask_c):
    maskcT = mask_c.T
    wlg = (1.0 / (1.0 + np.exp(-np.float64(gate_logit)))).astype(
        np.float32).reshape(1, 1)
    wlHi, wlLo = _hilo(np.ascontiguousarray(
        np.asarray(W_lang, np.float32).T) * np.float32(SW))
    wvoHi, wvoLo = _hilo(np.ascontiguousarray(Wvo.T) * np.float32(SW))

    # quantize h once per batch, in both layouts, then permute per core
    hq = []
    for b in range(B):
        hb = np.asarray(h[b], np.float32)
        rhi, rlo = _hilo(hb)                       # row layout [N, D]
        thi, tlo = _hilo(np.ascontiguousarray(hb.T))  # col layout [D, N]
        hq.append((rhi, rlo, thi, tlo))

    in_maps = []
    for core in range(8):
        b = core // 2
        pm = _posmap(core)
        rhi, rlo, thi, tlo = hq[b]
        mt = np.empty((NSLOT, 512, OWNW), np.float32)
        for s in range(NSLOT):
            mrows = pm[EXT[s] - 512:EXT[s]]
            ncols = pm[EXT[s] - OWNW:EXT[s]]
            mt[s] = maskcT[np.ix_(mrows, ncols)]
        in_maps.append({
            "htHi": np.ascontiguousarray(thi[:, pm]),
            "htLo": np.ascontiguousarray(tlo[:, pm]),
            "hrHi": np.ascontiguousarray(rhi[pm, :]),
            "hrLo": np.ascontiguousarray(rlo[pm, :]),
            "wlHi": wlHi, "wlLo": wlLo,
            "wvoHi": wvoHi, "wvoLo": wvoLo,
            "maskT": mt.astype(ml_dtypes.bfloat16), "wlg": wlg,
        })
    return in_maps


def _mask_fits_causal_tiling(mask_c):
    """True iff the mask is zero outside each block's processed extent and
    one everywhere in the unmasked interior the device skips."""
    for j in range(8):
        p = 0 if j % 2 == 1 else 1
        pm = _posmap(p)
        e = 256 * (j + 1) if p == 0 else 256 * (j + 2)
        rows = slice(256 * j, 256 * j + 256)
        if e < N and mask_c[rows, :][:, pm[e:]].any():
            return False
        interior = mask_c[rows, :][:, pm[:e - 512]]
        if (interior != 1.0).any():
            return False
    return True


def _kernel_numpy(h, causal_mask, W_lang, W_grav, W_V, W_O, gate_logit,
                  log_sigma):
    """Plain-numpy fallback mirroring the reference."""
    h = np.asarray(h, np.float32)
    mask = np.asarray(causal_mask, np.float32)
    not_eye = 1.0 - np.eye(N, dtype=np.float32)
    z_l = h @ np.asarray(W_lang, np.float32).T
    z_g = h @ np.asarray(W_grav, np.float32).T
    v = h @ np.asarray(W_V, np.float32).T
    zn = z_l / np.maximum(np.linalg.norm(z_l, axis=-1, keepdims=True), EPS)
    A_l = np.maximum(np.einsum("bnd,bmd->bnm", zn, zn), 0.0) * not_eye
    sq = (z_g * z_g).sum(-1, keepdims=True)
    d2 = np.maximum(sq + np.swapaxes(sq, -1, -2)
                    - 2.0 * np.einsum("bnd,bmd->bnm", z_g, z_g), 0.0)
    sigma = np.exp(np.float32(log_sigma))
    A_g = np.exp(-d2 / (2.0 * sigma * sigma)) * not_eye

    def norm(A):
        A = A * mask
        deg = np.maximum(A.sum(-1, keepdims=True), EPS)
        return A / deg

    w_l = 1.0 / (1.0 + np.exp(-np.float32(gate_logit)))
    K = w_l * norm(A_l) + (1.0 - w_l) * norm(A_g)
    out = np.einsum("bnm,bmd->bnd", K, v)
    return (out @ np.asarray(W_O, np.float32).T).astype(np.float32)


def kernel(h, causal_mask, W_lang, W_grav, W_V, W_O, gate_logit, log_sigma):
    mask_c = (np.asarray(causal_mask, np.float32)
              * (1.0 - np.eye(N, dtype=np.float32)))
    if not _mask_fits_causal_tiling(mask_c) or not _g_path_is_zero(
            h, W_grav, log_sigma, mask_c):
        return _kernel_numpy(h, causal_mask, W_lang, W_grav, W_V, W_O,
                             gate_logit, log_sigma)
    Wvo = (np.asarray(W_O, np.float32) @ np.asarray(W_V, np.float32))
    in_maps = _make_in_maps(h, W_lang, Wvo, gate_logit, mask_c)
    nc = _get_program()
    res = run_bass_kernel_spmd(nc, in_maps, core_ids=list(range(8)),
                               trace=TRACE)
    LAST_RESULTS[0] = res

    y = np.empty((B, N, D), np.float32)
    for core in range(8):
        b = core // 2
        pm = _posmap(core)
        yT = np.asarray(res.results[core]["yT"], dtype=np.float32)
        for s in range(NSLOT):
            rows = pm[EXT[s] - OWNW:EXT[s]]
            y[b, rows, :] = yT[:, s * OWNW:(s + 1) * OWNW].T
    return y


# revision 21
# speedup vs baseline: 1.0011x; 1.0011x over previous
"""DualLaplacianBlock Trainium2 kernel (v4 — slot-major fused pipeline).

Same math/host contract as v3 (fp8 DoubleRow hi/lo chains, K_g==0
specialization, W_O@W_V fusion, (batch, parity) sharding), but the device
program is restructured into a slot-major software pipeline so the PE never
waits on the vector engines:

    c0 G3 c1 G2 c2 G1 c3 G0 KV3 KV2 P4(pr1) KV1 KV0 P4(pr0)

where c<k> is a 512-column z-projection chunk, G<s> the slot-s gram (+deg),
KV<s> = K@h for slot s, and P4 the Wvo^T output GEMM per column half.  Each
slot's normalize/combine/cast work (V<s>) runs on DVE/Act/Pool in the shadow
of the next PE phase; the per-slot deg tail (last two m-tiles) is deferred
into the following PE phase to keep the PE queue from stalling on relu/mask
latency.
"""

import sys

if "/opt/trn_rl_repo" not in sys.path:
    sys.path.insert(0, "/opt/trn_rl_repo")

from contextlib import ExitStack

import ml_dtypes
import numpy as np

import concourse.bass as bass
import concourse.tile as tile
from concourse import bacc, mybir
from concourse.bass_utils import run_bass_kernel_spmd
from concourse.masks import make_identity

F32 = mybir.dt.float32
F32R = mybir.dt.float32r
BF16 = mybir.dt.bfloat16
FP8 = mybir.dt.float8e4
FP8L = mybir.dt.float8e5
AF = mybir.ActivationFunctionType
OP = mybir.AluOpType
PM = mybir.MatmulPerfMode

E4NP = ml_dtypes.float8_e4m3
E5NP = ml_dtypes.float8_e5m2

B, N, D = 4, 2048, 1024
P = 128
ET = D // P                      # 8 e-tiles (also d-tiles)
NSLOT = 4
EXT = [2048, 1536, 1024, 512]    # slot column extents (pattern, all cores)
MT = [e // P for e in EXT]       # m-tiles per slot: 16, 12, 8, 4
OFF = [0, 16, 28, 36]            # slot plane offsets in the K tiles
OWNW = 256                       # own columns per slot
EPS = 1e-8
SW = 32.0                        # host weight pre-scale (power of two)

TRACE = False          # set by test.py for profiling runs
LAST_RESULTS = [None]  # BassKernelResults stash for test.py


def _build_program():
    nc = bacc.Bacc("TRN2", target_bir_lowering=False, debug=False, num_devices=8)

    htHi_d = nc.dram_tensor("htHi", [D, N], FP8, kind="ExternalInput")
    htLo_d = nc.dram_tensor("htLo", [D, N], FP8, kind="ExternalInput")
    hrHi_d = nc.dram_tensor("hrHi", [N, D], FP8, kind="ExternalInput")
    hrLo_d = nc.dram_tensor("hrLo", [N, D], FP8, kind="ExternalInput")
    wlHi_d = nc.dram_tensor("wlHi", [D, D], FP8, kind="ExternalInput")
    wlLo_d = nc.dram_tensor("wlLo", [D, D], FP8, kind="ExternalInput")
    wvoHi_d = nc.dram_tensor("wvoHi", [D, D], FP8, kind="ExternalInput")
    wvoLo_d = nc.dram_tensor("wvoLo", [D, D], FP8, kind="ExternalInput")
    maskT_d = nc.dram_tensor("maskT", [NSLOT, 512, OWNW], BF16, kind="ExternalInput")
    wlg_d = nc.dram_tensor("wlg", [1, 1], F32, kind="ExternalInput")
    yT_d = nc.dram_tensor("yT", [D, 4 * OWNW], BF16, kind="ExternalOutput")

    def dview(t):  # [R, C] dram -> [128, R//128, C] view
        return t[:].rearrange("(o p) c -> p o c", p=P)

    with tile.TileContext(nc) as tc, ExitStack() as ctx:
        glob = ctx.enter_context(tc.tile_pool(name="glob", bufs=1))

        wl = glob.tile([1, 1], F32, tag="wl")
        wlb = glob.tile([P, 1], F32, tag="wlb")  # wl on all partitions

        onesf = glob.tile([P, 1], F32, tag="onesf")
        nc.vector.memset(onesf[:], 1.0)
        ones = glob.tile([P, 1], F32R, tag="ones")
        nc.scalar.activation(ones[:], onesf[:], AF.Copy)
        onesp = glob.tile([P, 2, P], FP8, tag="onesp")  # deg stationary
        nc.vector.memset(onesp[:], 1.0)
        ident = glob.tile([P, P], F32, tag="ident")
        make_identity(nc, ident[:])

        diagl = glob.tile([P, 16], F32, tag="diagl")  # |z~_m|^2 per m-tile
        rsl = glob.tile([P, 16], F32, tag="rsl")      # 1/|z~_m|
        msk = glob.tile([P, 16, OWNW], BF16, tag="msk")

        junkp = ctx.enter_context(tc.tile_pool(name="junkp", bufs=2))
        smp = ctx.enter_context(tc.tile_pool(name="smp", bufs=2))
        ypool = ctx.enter_context(tc.tile_pool(name="ypool", bufs=4))

        # big SBUF operands
        wpool = ctx.enter_context(tc.tile_pool(name="wpool", bufs=1))
        wlHi = wpool.tile([P, ET, D], FP8, tag="wHi")
        wlLo = wpool.tile([P, ET, D], FP8, tag="wLo")
        hpool = ctx.enter_context(tc.tile_pool(name="hpool", bufs=1))
        htHi = hpool.tile([P, ET, N], FP8, tag="htHi")
        htLo = hpool.tile([P, ET, N], FP8, tag="htLo")
        zpool = ctx.enter_context(tc.tile_pool(name="zpool", bufs=1))
        zlHi = zpool.tile([P, ET, N], FP8, tag="zlHi")
        zlLo = zpool.tile([P, ET, N], FP8, tag="zlLo")
        hrp = ctx.enter_context(tc.tile_pool(name="hrp", bufs=1))
        hrHi = hrp.tile([P, 16, D], FP8, tag="hrHi")
        hrLo = hrp.tile([P, 16, D], FP8, tag="hrLo")
        apool = ctx.enter_context(tc.tile_pool(name="apool", bufs=1))
        A_even = apool.tile([P, 16, OWNW], F32R, tag="Aeven")  # slots 0, 2
        A_odd = apool.tile([P, 12, OWNW], F32R, tag="Aodd")    # slots 1, 3
        kpool = ctx.enter_context(tc.tile_pool(name="kpool", bufs=1))
        kHi = kpool.tile([P, 40, OWNW], FP8, tag="kHi")
        kLo = kpool.tile([P, 40, OWNW], FP8L, tag="kLo")
        opool = ctx.enter_context(tc.tile_pool(name="opool", bufs=1))
        o2Hi = opool.tile([P, ET, 4 * OWNW], FP8, tag="o2Hi")
        o2Lo = opool.tile([P, ET, 4 * OWNW], FP8, tag="o2Lo")

        # PSUM: 2 (big: pz/pd/py) + 3 (pkv) + 2 (pg) + 1 (pdl) = 8 banks.
        # pg/pdl are allocated last (top of the PSUM stack) so they can be
        # released after the last gram/deg use and their banks recycled for
        # the dedicated P4(pr=0) pipeline pool.
        bigp = ctx.enter_context(tc.tile_pool(name="bigp", bufs=2, space="PSUM"))
        pkvp = ctx.enter_context(tc.tile_pool(name="pkvp", bufs=3, space="PSUM"))
        gram_psum = ExitStack()
        pgp = gram_psum.enter_context(tc.tile_pool(name="pgp", bufs=2, space="PSUM"))
        pdlp = gram_psum.enter_context(tc.tile_pool(name="pdlp", bufs=1, space="PSUM"))

        def A_of(s):
            return A_even if s % 2 == 0 else A_odd

        # ---------------- input DMA stream (consumption order) -----------
        h0, h1 = slice(0, 512), slice(512, 1024)
        c4s = [slice(c * 512, (c + 1) * 512) for c in range(4)]
        nc.sync.dma_start(wlHi[:, 0:2, h0], dview(wlHi_d)[:, 0:2, h0])
        nc.sync.dma_start(htHi[:, :, c4s[0]], dview(htHi_d)[:, :, c4s[0]])
        nc.sync.dma_start(wlHi[:, 2:8, h0], dview(wlHi_d)[:, 2:8, h0])
        nc.sync.dma_start(wlLo[:, :, h0], dview(wlLo_d)[:, :, h0])
        nc.sync.dma_start(htLo[:, :, c4s[0]], dview(htLo_d)[:, :, c4s[0]])
        nc.sync.dma_start(wlHi[:, :, h1], dview(wlHi_d)[:, :, h1])
        nc.sync.dma_start(wlLo[:, :, h1], dview(wlLo_d)[:, :, h1])
        nc.sync.dma_start(htHi[:, :, c4s[1]], dview(htHi_d)[:, :, c4s[1]])
        nc.sync.dma_start(htLo[:, :, c4s[1]], dview(htLo_d)[:, :, c4s[1]])
        nc.sync.dma_start(htHi[:, :, c4s[2]], dview(htHi_d)[:, :, c4s[2]])
        nc.sync.dma_start(htLo[:, :, c4s[2]], dview(htLo_d)[:, :, c4s[2]])
        nc.sync.dma_start(htHi[:, :, c4s[3]], dview(htHi_d)[:, :, c4s[3]])
        nc.sync.dma_start(htLo[:, :, c4s[3]], dview(htLo_d)[:, :, c4s[3]])
        nc.sync.dma_start(wl[:], wlg_d[:])
        nc.gpsimd.partition_broadcast(wlb[:], wl[:])
        nc.sync.dma_start(
            msk[:], maskT_d[:].rearrange("s (t p) n -> p (s t) n", p=P))
        nc.sync.dma_start(hrHi[:], dview(hrHi_d))
        nc.sync.dma_start(hrLo[:], dview(hrLo_d))
        # wvo loads are emitted after P1c3 (tag reuse of the wl slots)

        # deferred deg-matmul closures (flushed into the next PE phase)
        pending_degs = []

        def flush_degs():
            while pending_degs:
                pending_degs.pop(0)()

        # ---------------- phase-1 chunk: z projection + diag --------------
        def emit_p1_pair01(c):
            # head special-case: run the (wlHi, htHi) chains of et0 AND et1
            # before et0's Lo chains, so the PE has work while the wlLo /
            # htLo DMAs are still in flight
            cs = c4s[c]
            chains = ((wlHi, htHi), (wlLo, htHi), (wlHi, htLo))
            pzs = [bigp.tile([P, 512], F32, tag="big", name=f"pz01_{e}")
                   for e in range(2)]
            for (ci, et) in ((0, 0), (0, 1), (1, 0), (2, 0), (1, 1), (2, 1)):
                wa, hb = chains[ci]
                es = slice(et * P, (et + 1) * P)
                for dp in range(4):
                    nc.tensor.matmul(
                        pzs[et][:], wa[:, 2 * dp:2 * dp + 2, es],
                        hb[:, 2 * dp:2 * dp + 2, cs],
                        start=(ci == 0 and dp == 0),
                        stop=(ci == 2 and dp == 3),
                        perf_mode=PM.DoubleRow,
                        skip_group_check=True)
            for et in range(2):
                nc.scalar.copy(zlHi[:, et, cs], pzs[et][:])
                nc.vector.tensor_sub(zlLo[:, et, cs], pzs[et][:],
                                     zlHi[:, et, cs])

        def emit_p1_et(c, et):
            cs = c4s[c]
            es = slice(et * P, (et + 1) * P)
            pz = bigp.tile([P, 512], F32, tag="big")
            chains = ((wlHi, htHi), (wlLo, htHi), (wlHi, htLo))
            for ci, (wa, hb) in enumerate(chains):
                for dp in range(4):
                    nc.tensor.matmul(
                        pz[:], wa[:, 2 * dp:2 * dp + 2, es],
                        hb[:, 2 * dp:2 * dp + 2, cs],
                        start=(ci == 0 and dp == 0),
                        stop=(ci == 2 and dp == 3),
                        perf_mode=PM.DoubleRow,
                        skip_group_check=True)
            nc.scalar.copy(zlHi[:, et, cs], pz[:])
            nc.vector.tensor_sub(zlLo[:, et, cs], pz[:], zlHi[:, et, cs])

        def emit_p1_diag(c):
            for mt4 in range(4):
                gmt = c * 4 + mt4
                ms = slice(gmt * P, (gmt + 1) * P)
                pd = bigp.tile([P, 512], F32, tag="big")
                chains = ((zlHi, zlHi), (zlLo, zlHi), (zlHi, zlLo))
                for ci, (za, zb) in enumerate(chains):
                    for ep in range(4):
                        e2 = slice(2 * ep, 2 * ep + 2)
                        nc.tensor.matmul(
                            pd[:, 0:P], za[:, e2, ms], zb[:, e2, ms],
                            start=(ci == 0 and ep == 0),
                            stop=(ci == 2 and ep == 3),
                            perf_mode=PM.DoubleRow,
                            skip_group_check=True)
                junk = junkp.tile([P, P], F32, tag="junk")
                nc.vector.tensor_mul(junk[:], pd[:, 0:P], ident[:])
                nc.vector.reduce_sum(diagl[:, gmt:gmt + 1], junk[:],
                                     axis=mybir.AxisListType.X)
            cc = slice(c * 4, c * 4 + 4)
            nc.scalar.activation(rsl[:, cc], diagl[:, cc], AF.Sqrt)
            nc.vector.tensor_scalar(rsl[:, cc], rsl[:, cc], SW * EPS, None,
                                    OP.max)
            nc.vector.reciprocal(rsl[:, cc], rsl[:, cc])
            # extra 1/32: the A tiles stay unnormalized until the KV psum
            # rescale, so pull them into fp8e4m3 range here (the global
            # factor cancels in row normalization; dl's constant restores
            # the absolute output scale)
            nc.vector.tensor_scalar(rsl[:, cc], rsl[:, cc], 1.0 / 32.0, None,
                                    OP.mult)

        # ---------------- slot gram + cast + deg ---------------------------
        # A is cast to fp8 hi/lo UNNORMALIZED right after relu/mask (the
        # 1/32 in rsl keeps it in e4m3 range); row normalization is applied
        # later as a single psum rescale per KV group.  deg accumulates from
        # the fp8 casts with DoubleRow pairs (half the PE time of the f32r
        # row-sum, and it frees the A f32 tiles early).
        def emit_G(s):
            at = A_of(s)
            own = slice(EXT[s] - OWNW, EXT[s])
            pdl = pdlp.tile([P, OWNW], F32, tag="pdl", name=f"pdl{s}")

            def emit_deg(g0):
                ks = slice(OFF[s] + g0, OFF[s] + g0 + 2)
                for ci, src in enumerate((kHi, kLo)):
                    nc.tensor.matmul(
                        pdl[:], onesp[:, 0:2, 0:P], src[:, ks, :],
                        start=(g0 == 0 and ci == 0),
                        stop=(g0 == MT[s] - 2 and ci == 1),
                        perf_mode=PM.DoubleRow,
                        skip_group_check=True)

            for gmt in range(MT[s]):
                ms = slice(gmt * P, (gmt + 1) * P)
                pg = pgp.tile([P, OWNW], F32, tag="pg")
                chains = ((zlHi, zlHi), (zlLo, zlHi), (zlHi, zlLo))
                for ci, (za, zb) in enumerate(chains):
                    for ep in range(4):
                        e2 = slice(2 * ep, 2 * ep + 2)
                        nc.tensor.matmul(
                            pg[:], za[:, e2, ms], zb[:, e2, own],
                            start=(ci == 0 and ep == 0),
                            stop=(ci == 2 and ep == 3),
                            perf_mode=PM.DoubleRow,
                            skip_group_check=True)
                nc.scalar.activation(at[:, gmt, :], pg[:], AF.Relu,
                                     scale=rsl[:, gmt:gmt + 1])
                if gmt >= MT[s] - 4:
                    bi = 4 * s + gmt - (MT[s] - 4)
                    nc.gpsimd.tensor_mul(at[:, gmt, :], at[:, gmt, :],
                                         msk[:, bi, :])
                if gmt % 2 == 1:
                    g0 = gmt - 1
                    ks = slice(OFF[s] + g0, OFF[s] + g0 + 2)
                    asl = slice(g0, g0 + 2)
                    nc.scalar.copy(kHi[:, ks, :], at[:, asl, :])
                    nc.vector.tensor_sub(kLo[:, ks, :], at[:, asl, :],
                                         kHi[:, ks, :])
                    if g0 >= 2:
                        emit_deg(g0 - 2)
            pending_degs.append(lambda g=MT[s] - 2: emit_deg(g))
            return pdl

        # ---------------- slot normalize (dinv broadcast only) ------------
        def emit_V(s, pdl):
            dlb = smp.tile([P, OWNW], F32, tag=f"dlb{s}", name=f"dlb{s}")
            nc.vector.tensor_scalar(dlb[:], pdl[:], EPS, None, OP.max)
            nc.vector.reciprocal(dlb[:], dlb[:])
            nc.vector.tensor_scalar(dlb[:], dlb[:], wlb[:], SW, OP.mult,
                                    OP.mult)
            return dlb

        # ---------------- slot KV: o2[:, :, scol] = (K_s @ h)^T -----------
        def emit_KV(s, dlb, flush_after=None):
            scol = slice(s * OWNW, (s + 1) * OWNW)
            for eh in range(2):
                for e2 in range(4):
                    et = eh * 4 + e2
                    es = slice(et * P, (et + 1) * P)
                    pkv = pkvp.tile([P, OWNW], F32, tag="pkv")
                    chains = ((hrHi, kHi), (hrLo, kHi), (hrHi, kLo))
                    for ci, (ha, kt) in enumerate(chains):
                        for gp in range(0, MT[s], 2):
                            nc.tensor.matmul(
                                pkv[:], ha[:, gp:gp + 2, es],
                                kt[:, OFF[s] + gp:OFF[s] + gp + 2, :],
                                start=(ci == 0 and gp == 0),
                                stop=(ci == 2 and gp == MT[s] - 2),
                                perf_mode=PM.DoubleRow,
                                skip_group_check=True)
                    # row normalization: one psum rescale per group
                    nc.vector.tensor_mul(pkv[:], pkv[:], dlb[:])
                    nc.scalar.copy(o2Hi[:, et, scol], pkv[:])
                    nc.vector.tensor_sub(o2Lo[:, et, scol], pkv[:],
                                         o2Hi[:, et, scol])
                    if flush_after is not None and eh == 0 and e2 == flush_after:
                        flush_degs()

        # ---------------- output half: y = out2 @ Wvo^T -------------------
        def emit_P4(pr, wvoHi, wvoLo, pool=None):
            # pr=0 runs with a dedicated 3-buffer pool (recycled pg/pdl
            # banks) in 256-wide groups: deeper psum pipelining and a short
            # drain for the kernel tail; y DMAs stay 512 wide.
            fine = pool is not None
            w = 256 if fine else 512
            grp = 1   # e2t planes per y DMA
            oc2 = slice(pr * 512, (pr + 1) * 512)
            yt = None
            for e2t in range(ET):
                if e2t % grp == 0:
                    yt = ypool.tile([P, grp, 512], BF16, tag=f"yt{grp}")
                for half in range(512 // w):
                    ocol = slice(pr * 512 + half * w, pr * 512 + (half + 1) * w)
                    if fine:
                        py = pool.tile([P, w], F32, tag="pyf",
                                       name=f"py{pr}_{e2t}_{half}")
                    else:
                        py = bigp.tile([P, w], F32, tag="big",
                                       name=f"py{pr}_{e2t}")
                    chains = ((wvoHi, o2Hi), (wvoLo, o2Hi), (wvoHi, o2Lo))
                    for ci, (wa, ob) in enumerate(chains):
                        for dp in range(4):
                            d2 = slice(2 * dp, 2 * dp + 2)
                            nc.tensor.matmul(
                                py[:], wa[:, d2, e2t * P:(e2t + 1) * P],
                                ob[:, d2, ocol],
                                start=(ci == 0 and dp == 0),
                                stop=(ci == 2 and dp == 3),
                                perf_mode=PM.DoubleRow,
                                skip_group_check=True)
                    nc.scalar.mul(
                        yt[:, e2t % grp, half * w:(half + 1) * w], py[:],
                        1.0 / (32.0 * SW))
                if e2t % grp == grp - 1:
                    e0 = e2t - grp + 1
                    nc.sync.dma_start(
                        dview(yT_d)[:, e0:e2t + 1, oc2], yt[:])

        # ================= emission sequence ==============================
        emit_p1_pair01(0)
        for et in range(2, ET):
            emit_p1_et(0, et)
        emit_p1_diag(0)

        pdl3 = emit_G(3)

        for et in range(ET):
            emit_p1_et(1, et)
            if et == 0:
                flush_degs()
        emit_p1_diag(1)
        dlb3 = emit_V(3, pdl3)

        pdl2 = emit_G(2)

        for et in range(ET):
            emit_p1_et(2, et)
            if et == 0:
                flush_degs()
        emit_p1_diag(2)
        dlb2 = emit_V(2, pdl2)

        pdl1 = emit_G(1)

        for et in range(ET):
            emit_p1_et(3, et)
            if et == 0:
                flush_degs()
        emit_p1_diag(3)
        dlb1 = emit_V(1, pdl1)

        pdl0 = emit_G(0)

        # wl slots are dead after P1c3: reuse them for wvo (WAR-tracked)
        wvoHi = wpool.tile([P, ET, D], FP8, tag="wHi", name="wvoHi")
        nc.sync.dma_start(wvoHi[:], dview(wvoHi_d))
        wvoLo = wpool.tile([P, ET, D], FP8, tag="wLo", name="wvoLo")
        nc.sync.dma_start(wvoLo[:], dview(wvoLo_d))

        emit_KV(3, dlb3, flush_after=1)
        dlb0 = emit_V(0, pdl0)
        emit_KV(2, dlb2)
        emit_P4(1, wvoHi, wvoLo)
        emit_KV(1, dlb1)
        emit_KV(0, dlb0)
        gram_psum.close()  # pg/pdl banks -> P4(pr=0) pipeline pool
        pyfp = ctx.enter_context(tc.tile_pool(name="pyfp", bufs=3,
                                              space="PSUM"))
        emit_P4(0, wvoHi, wvoLo, pool=pyfp)

    nc.compile()
    return nc


_PROGRAM = None


def _get_program():
    global _PROGRAM
    if _PROGRAM is None:
        _PROGRAM = _build_program()
    return _PROGRAM


def _posmap(core):
    """Device position -> global sequence row for this core."""
    p = core % 2
    q = np.arange(N)
    if p == 0:
        return q
    return (q // 512) * 512 + (q % 512 + 256) % 512


def _hilo(x, lot=E4NP):
    hi = np.asarray(x).astype(E4NP)
    lo = (x - hi.astype(np.float32)).astype(lot)
    return hi, lo


def _g_path_is_zero(h, W_grav, log_sigma, mask_c):
    """Exact f32 check that exp(-d2/(2 sigma^2)) == 0 for all masked pairs."""
    sigma = np.exp(np.float32(log_sigma)).astype(np.float32)
    thresh = np.float32(-110.0) * (2.0 * sigma * sigma)
    WgT = np.asarray(W_grav, np.float32).T
    for b in range(B):
        zg = np.asarray(h[b], np.float32) @ WgT
        sq = np.einsum("nd,nd->n", zg, zg)
        d2 = sq[:, None] + sq[None, :] - 2.0 * (zg @ zg.T)
        if (d2[mask_c > 0] + thresh < 0).any():
            return False
    return True


def _make_in_maps(h, W_lang, Wvo, gate_logit, mask_c):
    maskcT = mask_c.T
    wlg = (1.0 / (1.0 + np.exp(-np.float64(gate_logit)))).astype(
        np.float32).reshape(1, 1)
    wlHi, wlLo = _hilo(np.ascontiguousarray(
        np.asarray(W_lang, np.float32).T) * np.float32(SW))
    wvoHi, wvoLo = _hilo(np.ascontiguousarray(Wvo.T) * np.float32(SW))

    # quantize h once per batch, in both layouts, then permute per core
    hq = []
    for b in range(B):
        hb = np.asarray(h[b], np.float32)
        rhi, rlo = _hilo(hb)                       # row layout [N, D]
        thi, tlo = _hilo(np.ascontiguousarray(hb.T))  # col layout [D, N]
        hq.append((rhi, rlo, thi, tlo))

    in_maps = []
    for core in range(8):
        b = core // 2
        pm = _posmap(core)
        rhi, rlo, thi, tlo = hq[b]
        mt = np.empty((NSLOT, 512, OWNW), np.float32)
        for s in range(NSLOT):
            mrows = pm[EXT[s] - 512:EXT[s]]
            ncols = pm[EXT[s] - OWNW:EXT[s]]
            mt[s] = maskcT[np.ix_(mrows, ncols)]
        in_maps.append({
            "htHi": np.ascontiguousarray(thi[:, pm]),
            "htLo": np.ascontiguousarray(tlo[:, pm]),
            "hrHi": np.ascontiguousarray(rhi[pm, :]),
            "hrLo": np.ascontiguousarray(rlo[pm, :]),
            "wlHi": wlHi, "wlLo": wlLo,
            "wvoHi": wvoHi, "wvoLo": wvoLo,
            "maskT": mt.astype(ml_dtypes.bfloat16), "wlg": wlg,
        })
    return in_maps


def _mask_fits_causal_tiling(mask_c):
    """True iff the mask is zero outside each block's processed extent and
    one everywhere in the unmasked interior the device skips."""
    for j in range(8):
        p = 0 if j % 2 == 1 else 1
        pm = _posmap(p)
        e = 256 * (j + 1) if p == 0 else 256 * (j + 2)
        rows = slice(256 * j, 256 * j + 256)
        if e < N and mask_c[rows, :][:, pm[e:]].any():
            return False
        interior = mask_c[rows, :][:, pm[:e - 512]]
        if (interior != 1.0).any():
            return False
    return True


def _kernel_numpy(h, causal_mask, W_lang, W_grav, W_V, W_O, gate_logit,
                  log_sigma):
    """Plain-numpy fallback mirroring the reference."""
    h = np.asarray(h, np.float32)
    mask = np.asarray(causal_mask, np.float32)
    not_eye = 1.0 - np.eye(N, dtype=np.float32)
    z_l = h @ np.asarray(W_lang, np.float32).T
    z_g = h @ np.asarray(W_grav, np.float32).T
    v = h @ np.asarray(W_V, np.float32).T
    zn = z_l / np.maximum(np.linalg.norm(z_l, axis=-1, keepdims=True), EPS)
    A_l = np.maximum(np.einsum("bnd,bmd->bnm", zn, zn), 0.0) * not_eye
    sq = (z_g * z_g).sum(-1, keepdims=True)
    d2 = np.maximum(sq + np.swapaxes(sq, -1, -2)
                    - 2.0 * np.einsum("bnd,bmd->bnm", z_g, z_g), 0.0)
    sigma = np.exp(np.float32(log_sigma))
    A_g = np.exp(-d2 / (2.0 * sigma * sigma)) * not_eye

    def norm(A):
        A = A * mask
        deg = np.maximum(A.sum(-1, keepdims=True), EPS)
        return A / deg

    w_l = 1.0 / (1.0 + np.exp(-np.float32(gate_logit)))
    K = w_l * norm(A_l) + (1.0 - w_l) * norm(A_g)
    out = np.einsum("bnm,bmd->bnd", K, v)
    return (out @ np.asarray(W_O, np.float32).T).astype(np.float32)


def kernel(h, causal_mask, W_lang, W_grav, W_V, W_O, gate_logit, log_sigma):
    mask_c = (np.asarray(causal_mask, np.float32)
              * (1.0 - np.eye(N, dtype=np.float32)))
    if not _mask_fits_causal_tiling(mask_c) or not _g_path_is_zero(
            h, W_grav, log_sigma, mask_c):
        return _kernel_numpy(h, causal_mask, W_lang, W_grav, W_V, W_O,
                             gate_logit, log_sigma)
    Wvo = (np.asarray(W_O, np.float32) @ np.asarray(W_V, np.float32))
    in_maps = _make_in_maps(h, W_lang, Wvo, gate_logit, mask_c)
    nc = _get_program()
    res = run_bass_kernel_spmd(nc, in_maps, core_ids=list(range(8)),
                               trace=TRACE)
    LAST_RESULTS[0] = res

    y = np.empty((B, N, D), np.float32)
    for core in range(8):
        b = core // 2
        pm = _posmap(core)
        yT = np.asarray(res.results[core]["yT"], dtype=np.float32)
        for s in range(NSLOT):
            rows = pm[EXT[s] - OWNW:EXT[s]]
            y[b, rows, :] = yT[:, s * OWNW:(s + 1) * OWNW].T
    return y
